# revision 43
# baseline (speedup 1.0000x reference)
"""Trainium2 Bass kernel for nn_CrossCompressUnit (rank-1 cross-compress unit).

Math (per row i of the [B, 128] inputs v, e):
    a_i = e_i . w_vv ; b_i = v_i . w_ev ; c_i = e_i . w_ve ; d_i = v_i . w_ee
    v_out_i = a_i * v_i + b_i * e_i + b_v
    e_out_i = c_i * v_i + d_i * e_i + b_e

Strategy: data-parallel over 8 NeuronCores (B/8 = 16384 rows per core),
fp16 I/O. The harness gate is rel_err < 2e-2; fp16 end-to-end measures
~7e-4, and it halves HBM traffic: 32 MiB/core (f32) -> 16 MiB/core, so
the DMA roofline drops from ~93.2us to ~46.6us.

Layout: the host hands each core v/e as [128, 16384] fp16 where
partition p, free (m, d) holds row p*128+m, feature d (a free reshape
of the row-major shard). Every [128, 128] block then has one row per
partition, so per-row dot products act as per-partition scalars, and
DRAM lines are 2 KiB-contiguous per partition (>= 512B keeps the DMA
cost model at full bus speed for 2-byte data).

Per 512-row group: PE transposes the 4 blocks of v and e (fp16: 1
cycle/row) into one PSUM bank; ScalarE copies [vT|eT] back to SBUF as
fp16 in a single 1024-wide op; PE matmuls vT/eT blocks against the
[128, 4] fp16 weight matrix giving all per-row dots with rows on
partitions; ScalarE copies dots PSUM->SBUF. Per block the elementwise
combine is 2 ops per output: ts VO = E*b, then the fused
stt VO = (V*a)+VO. Walrus only runs stt on DVE, so DVE carries all 256
stt (~194ns each), and the ts ops split 6 GPSIMD / 1 ScalarE / 1 DVE
per group, with the DVE slot rotating to ScalarE every 4th group (LP
optimum) -- DVE/GPSIMD/ScalarE all land at ~51-52us, just above the
46.6us DMA floor.

Schedule notes (timeline cost model):
  - A DMA's semaphore waits park on the issuing engine's SEQ, so output
    DMAs are issued on the otherwise-idle SP queue one chunk late
    (waits already satisfied); issuing them on ScalarE right after the
    producing stt ops starved the ScalarE engine ~2us per chunk.
  - PE is in-order: transposes of group g+1 are emitted before the dot
    matmuls of group g, otherwise PE idles behind the 1us ScalarE copy
    and every group serializes on the copy->mm->transpose->copy loop.
  - Input DMAs prefetch 2 chunks ahead (issue-to-ready ~3.8us).
  - The transpose identity is built on-device (memset + affine_select)
    instead of DMA'd: a DMA'd identity lands ~3.8us into the run behind
    the first data chunks on the shared HWDGE and gates every
    transpose during ramp.

Cost-model timeline: 65.6us/core vs baseline f32 kernel 97.9us.
"""

import os
import sys
from contextlib import ExitStack

import numpy as np

for _p in ("/root/.axon_site", "/root/.axon_site/_ro/trn_rl_repo",
           "/root/.axon_site/_ro/pypackages", "/opt/trn_rl_repo"):
    if os.path.isdir(_p) and _p not in sys.path:
        sys.path.append(_p)

import concourse.bass as bass
import concourse.tile as tile
from concourse import bacc, mybir
from concourse.bass_utils import run_bass_kernel_spmd

F32 = mybir.dt.float32
F16 = mybir.dt.float16

B, D = 131072, 128
N_CORES = 8
SHARD = B // N_CORES          # 16384 rows per core
FREE = SHARD                  # free elems per dram tensor row-block view
CHUNK = 1024                  # rows (= free elems) per DMA chunk
GROUP = 512                   # rows per compute group (4 blocks of 128)

# Elementwise op scheduling: per group there are 8 ts + 8 stt. Walrus
# only allows the fused stt on DVE, so DVE carries all stt (the
# bottleneck at ~193ns each) and the ts ops are pushed to GPSIMD and
# ScalarE to keep DVE as close to stt-only as possible.
TS_POOL_PER_GROUP = 6         # of 8 ts, how many on GPSIMD
TS_ACT_PER_GROUP = 1          # of 8 ts, how many on ScalarE (rest DVE)


def _emit(ctx, tc, vin, ein, vout, eout, w4, bvb, shard, has_bias):
    nc = tc.nc
    n_chunks = shard // CHUNK
    gpc = CHUNK // GROUP          # groups per chunk (2)
    n_groups = shard // GROUP
    MULT = mybir.AluOpType.mult
    ADD = mybir.AluOpType.add

    in_pool = ctx.enter_context(tc.tile_pool(name="in", bufs=4))
    out_pool = ctx.enter_context(tc.tile_pool(name="out", bufs=5))
    tsb_pool = ctx.enter_context(tc.tile_pool(name="tsb", bufs=3))
    ps_pool = ctx.enter_context(tc.tile_pool(name="ps", bufs=3, space="PSUM"))
    dps_pool = ctx.enter_context(tc.tile_pool(name="dps", bufs=2, space="PSUM"))
    const_pool = ctx.enter_context(tc.tile_pool(name="const", bufs=1))

    # Identity built on-device (memset + affine_select, ~0.5us): a DMA'd
    # identity arrives at ~3.8us behind the first data chunks on the
    # shared HWDGE and gates every transpose during ramp.
    id_t = const_pool.tile([128, 128], F16, tag="ident")
    nc.gpsimd.memset(id_t[:], 1.0)
    nc.gpsimd.affine_select(
        out=id_t[:], in_=id_t[:], compare_op=mybir.AluOpType.is_equal,
        fill=0.0, base=0, pattern=[[-1, 128]], channel_multiplier=1)
    w4_t = const_pool.tile([128, 4], F16, tag="w4")
    nc.scalar.dma_start(w4_t[:], w4[:, :])

    # Dummy PE consumers of the const tiles: walrus allows only one sync
    # wait per matmul, so PE absorbs the const-DMA semaphores here rather
    # than on the first real matmul (which already carries a data wait).
    junk = ps_pool.tile([128, 128], F16, tag="junk", bufs=1)
    nc.tensor.transpose(junk[:], id_t[:], id_t[:])
    junkm = dps_pool.tile([128, 32], F32, tag="junkm", bufs=1)
    nc.tensor.matmul(junkm[0:4, 0:4], w4_t[:], w4_t[:])

    if has_bias:
        bcat_t = const_pool.tile([128, 256], F16, tag="bcat")
        nc.scalar.dma_start(bcat_t[:], bvb[:, :])

    # All per-row dot products for the whole shard live here ([128, 32]
    # slice per group), written by GPSIMD, read by the elementwise ops.
    # No tile reuse -> no WAR semaphores.
    dots_all = const_pool.tile([128, 32 * n_groups], F32, tag="dots_all")

    # Software-pipelined emission (one group ahead on PE/ACT): the PE
    # stream must run transposes of group g+1 BEFORE the dot matmuls of
    # group g, otherwise PE idles behind the big ScalarE PSUM copy and
    # the group cycle serializes on the copy->mm->transpose->copy loop.
    V_t, E_t, VO_t, EO_t = {}, {}, {}, {}
    tT_t = {}

    def chunk_of(gi):
        return gi // gpc

    def ensure_in(c):
        if c in V_t or c >= n_chunks:
            return
        cs = slice(c * CHUNK, (c + 1) * CHUNK)
        V = in_pool.tile([128, CHUNK], F16, tag="V")
        E = in_pool.tile([128, CHUNK], F16, tag="E")
        if c == 0:
            # Halve the first chunk's transfers so the first transposes
            # (which only need the first 512 columns) start ~1.5us
            # earlier during pipeline ramp.
            for h in range(gpc):
                hs = slice(h * GROUP, (h + 1) * GROUP)
                hd = slice(c * CHUNK + h * GROUP, c * CHUNK + (h + 1) * GROUP)
                nc.sync.dma_start(V[:, hs], vin[:, hd])
                nc.sync.dma_start(E[:, hs], ein[:, hd])
        else:
            nc.sync.dma_start(V[:], vin[:, cs])
            nc.sync.dma_start(E[:], ein[:, cs])
        V_t[c], E_t[c] = V, E
        VO_t[c] = out_pool.tile([128, CHUNK], F16, tag="VO", name="VO")
        EO_t[c] = out_pool.tile([128, CHUNK], F16, tag="EO", name="EO")

    def trans_and_copy(gi):
        # PE: 8 transposes into one PSUM bank; ACT: one fused PSUM->SBUF
        # fp16 copy of [vT(512) | eT(512)].
        c, g = divmod(gi, gpc)
        ensure_in(c)
        V, E = V_t[c], E_t[c]
        t_ps = ps_pool.tile([128, 2 * GROUP], F16, tag="t_ps")
        for b in range(4):
            ks = slice((4 * g + b) * 128, (4 * g + b + 1) * 128)
            bs = slice(b * 128, (b + 1) * 128)
            es = slice(GROUP + b * 128, GROUP + (b + 1) * 128)
            nc.tensor.transpose(t_ps[:, bs], V[:, ks], id_t[:])
            nc.tensor.transpose(t_ps[:, es], E[:, ks], id_t[:])
        tT = tsb_pool.tile([128, 2 * GROUP], F16, tag="tT")
        nc.scalar.copy(tT[:], t_ps[:])
        tT_t[gi] = tT

    ts_ctr = [0]

    def group_body(gi):
        c, g = divmod(gi, gpc)
        V, E = V_t[c], E_t[c]
        VO, EO = VO_t[c], EO_t[c]
        tT = tT_t.pop(gi)

        # dotsT[r, j] = x_r . w_j ; w cols = (w_vv, w_ev, w_ve, w_ee)
        dots_ps = dps_pool.tile([128, 32], F32, tag="dots_ps")
        for b in range(4):
            bs = slice(b * 128, (b + 1) * 128)
            es = slice(GROUP + b * 128, GROUP + (b + 1) * 128)
            nc.tensor.matmul(dots_ps[:, b * 8:b * 8 + 4], tT[:, bs], w4_t[:])
            nc.tensor.matmul(dots_ps[:, b * 8 + 4:b * 8 + 8], tT[:, es],
                             w4_t[:])
        dots = dots_all[:, gi * 32:(gi + 1) * 32]
        nc.scalar.copy(dots[:], dots_ps[:])

        for b in range(4):
            ks = slice((4 * g + b) * 128, (4 * g + b + 1) * 128)
            a_ = dots[:, b * 8 + 4:b * 8 + 5]   # e . w_vv
            b_ = dots[:, b * 8 + 1:b * 8 + 2]   # v . w_ev
            c_ = dots[:, b * 8 + 6:b * 8 + 7]   # e . w_ve
            d_ = dots[:, b * 8 + 3:b * 8 + 4]   # v . w_ee

            for out_t, sc_e, sc_v in ((VO, b_, a_), (EO, d_, c_)):
                # out = E * sc_e ; out = (V * sc_v) + out
                i = ts_ctr[0] % 8
                if i < TS_POOL_PER_GROUP:
                    nc.gpsimd.tensor_scalar_mul(out_t[:, ks], E[:, ks], sc_e)
                elif i < TS_POOL_PER_GROUP + TS_ACT_PER_GROUP or (
                        ts_ctr[0] // 8) % 4 == 3:
                    # LP optimum is ~0.77 DVE-ts and ~1.28 ACT-ts per
                    # group: the 8th slot goes to ScalarE every 4th
                    # group instead of DVE.
                    nc.scalar.mul(out_t[:, ks], E[:, ks], sc_e)
                else:
                    nc.vector.tensor_scalar_mul(out_t[:, ks], E[:, ks], sc_e)
                ts_ctr[0] += 1
                nc.vector.scalar_tensor_tensor(out_t[:, ks], V[:, ks],
                                               sc_v, out_t[:, ks], MULT, ADD)
            if has_bias:
                nc.gpsimd.tensor_add(VO[:, ks], VO[:, ks], bcat_t[:, 0:128])
                nc.gpsimd.tensor_add(EO[:, ks], EO[:, ks], bcat_t[:, 128:256])

        if g == gpc - 1:
            del V_t[c], E_t[c]

    def emit_out(c, half=None):
        # Output DMAs go on the idle SP queue, emitted one chunk late:
        # a DMA's semaphore waits park on the issuing engine's SEQ, so
        # issuing right after the producing stt ops would block that
        # SEQ for the whole elementwise tail of the chunk.
        if half is None:
            cs = slice(c * CHUNK, (c + 1) * CHUNK)
            nc.sync.dma_start(vout[:, cs], VO_t.pop(c)[:])
            nc.sync.dma_start(eout[:, cs], EO_t.pop(c)[:])
        else:
            # Last chunk: per-group halves on both queues to shrink the
            # drain tail.
            hs = slice(half * GROUP, (half + 1) * GROUP)
            hd = slice(c * CHUNK + half * GROUP,
                       c * CHUNK + (half + 1) * GROUP)
            VO = VO_t[c] if half < gpc - 1 else VO_t.pop(c)
            EO = EO_t[c] if half < gpc - 1 else EO_t.pop(c)
            nc.sync.dma_start(vout[:, hd], VO[:, hs])
            nc.scalar.dma_start(eout[:, hd], EO[:, hs])

    ensure_in(0)
    ensure_in(1)
    ensure_in(2)
    trans_and_copy(0)
    for gi in range(n_groups):
        # Prefetch the input DMA ~2 chunks (4 group cycles) ahead: issue
        # to data-ready latency is ~3.8us, which one group cycle cannot
        # hide.
        ensure_in(gi // gpc + 2)
        if gi + 1 < n_groups:
            trans_and_copy(gi + 1)
        group_body(gi)
        c, g = divmod(gi, gpc)
        if g == 0 and c >= 1:
            emit_out(c - 1)
        if c == n_chunks - 1:
            emit_out(c, half=g)


def _build(shard, has_bias):
    # Bacc (not raw Bass): its compile() runs move_matmul_waits_to_ldweights
    # and generate_event_semaphores, which legalize the one-sync-wait-per-
    # instruction hardware constraint that walrus codegen enforces.
    nc = bacc.Bacc("TRN2", target_bir_lowering=False, debug=False)
    vin = nc.dram_tensor("v", [128, shard], F16, kind="ExternalInput").ap()
    ein = nc.dram_tensor("e", [128, shard], F16, kind="ExternalInput").ap()
    w4 = nc.dram_tensor("w4", [128, 4], F16, kind="ExternalInput").ap()
    bvb = None
    if has_bias:
        bvb = nc.dram_tensor("bvb", [128, 256], F16,
                             kind="ExternalInput").ap()
    vout = nc.dram_tensor("v_out", [128, shard], F16,
                          kind="ExternalOutput").ap()
    eout = nc.dram_tensor("e_out", [128, shard], F16,
                          kind="ExternalOutput").ap()
    with tile.TileContext(nc) as tc:
        with ExitStack() as ctx:
            _emit(ctx, tc, vin, ein, vout, eout, w4, bvb,
                  shard, has_bias)
    nc.compile()
    return nc


def _run(inputs, trace=False):
    v = np.asarray(inputs["v"], dtype=np.float32)
    e = np.asarray(inputs["e"], dtype=np.float32)
    w_vv = np.asarray(inputs["w_vv"], dtype=np.float32)
    w_ev = np.asarray(inputs["w_ev"], dtype=np.float32)
    w_ve = np.asarray(inputs["w_ve"], dtype=np.float32)
    w_ee = np.asarray(inputs["w_ee"], dtype=np.float32)
    b_v = np.asarray(inputs["b_v"], dtype=np.float32)
    b_e = np.asarray(inputs["b_e"], dtype=np.float32)

    has_bias = bool(np.any(b_v) or np.any(b_e))
    w4 = np.ascontiguousarray(
        np.stack([w_vv, w_ev, w_ve, w_ee], axis=1)).astype(np.float16)

    v16 = np.ascontiguousarray(v.astype(np.float16))
    e16 = np.ascontiguousarray(e.astype(np.float16))

    nc = _build(SHARD, has_bias)

    in_maps = []
    for i in range(N_CORES):
        # [SHARD, 128] row-major == [128, SHARD] with row p*128+m at
        # partition p, free m*128+d: a pure reshape, no copy.
        m = {
            "v": v16[i * SHARD:(i + 1) * SHARD].reshape(128, SHARD),
            "e": e16[i * SHARD:(i + 1) * SHARD].reshape(128, SHARD),
            "w4": w4,
        }
        if has_bias:
            m["bvb"] = np.ascontiguousarray(np.concatenate([
                np.tile(b_v[None, :], (128, 1)),
                np.tile(b_e[None, :], (128, 1))], axis=1)).astype(np.float16)
        in_maps.append(m)

    res = run_bass_kernel_spmd(nc, in_maps, list(range(N_CORES)), trace=trace)
    v_out = np.concatenate(
        [res.results[i]["v_out"].astype(np.float32).reshape(SHARD, D)
         for i in range(N_CORES)], 0)
    e_out = np.concatenate(
        [res.results[i]["e_out"].astype(np.float32).reshape(SHARD, D)
         for i in range(N_CORES)], 0)
    return (v_out, e_out), res


def kernel(**inputs):
    out, _ = _run(inputs, trace=False)
    return out
